# revision 20
# baseline (speedup 1.0000x reference)
"""Causal single-head attention (B=4, T=4096, C=1024, H=128) on 8 Trainium2
NeuronCores.

Sharding: 8 cores = 4 batches x 2 key-parity shards. Each core handles one
batch and the keys in every other 128-block (parity h = core % 2), computing
UN-normalized partial attention (numerator O^T and per-key exp sums) for ALL
4096 queries of its batch via unsafe softmax (scores are N(0,1)-bounded,
|s| < 10, so exp never overflows and max-subtraction is unnecessary; partial
results combine exactly by summation across the two cores of a batch).

SPMD uniformity trick: the program is identical on all cores; per-core
differences live entirely in the DATA. The host passes x[b].T with its
columns block-permuted so that this core's keys always sit at the EVEN
128-block positions, plus per-core causal mask tiles for the diagonal
blocks of each 512-query group. The host un-permutes the returned query
axis and combines: out = (O0 + O1) / (d0 + d1), transposed.

v2 (bf16): all SBUF operands are bf16 (PSUM accumulation stays f32):
  - halves x DMA traffic and SBUF footprint
  - DVE tensor ops run in 2x_1p mode (0.5 cyc/elem)
  - S psum tiles are [128, 1024] (2 banks, 2 key blocks): half the ACT
    instructions, one full-tile DVE accumulate per tile
  - the softmax denominator is NOT reduced on-chip: the per-group
    accumulated exp tile eacc [128, 1024] is DMA'd out raw and the host
    does the final 128x2-way sum (removes all ones-matmuls + [1,512]
    copies from the hot loop)
  - V^T->V transposes write 4 sub-blocks into ONE psum bank, one copy out

On-chip layout:
  S^T halves [128 keys, 512 queries] = matmul(lhsT=K^T block, rhs=Q^T group)
  E = exp(S^T) on ACT (PSUM->SBUF, bf16), diagonal tile masked on DVE
  O^T += matmul(lhsT=V block [k,H], rhs=E half)   accumulated in PSUM f32
  eacc += E on DVE (bf16), shipped to host per group
"""

import sys
import numpy as np

sys.path.insert(0, "/opt/trn_rl_repo")

B, T, C, H = 4, 4096, 1024, 128
KB = 128            # key block
QG = 512            # query group
NKB = T // KB       # 32 global key blocks
NQG = T // QG       # 8 query groups
NCH = C // 128      # 8 contraction chunks
NST = 4             # supertiles of 1024 positions
SCALE = float(H) ** -0.5

_prog_cache = {}


def _build_program():
    import concourse.mybir as mybir
    import concourse.tile as tile
    from concourse import bacc

    F32 = mybir.dt.float32
    BF16 = mybir.dt.bfloat16
    AF = mybir.ActivationFunctionType

    nc = bacc.Bacc()
    xt = nc.dram_tensor("xt", [C, T], BF16, kind="ExternalInput")
    wq = nc.dram_tensor("wq", [C, H], BF16, kind="ExternalInput")
    wk = nc.dram_tensor("wk", [C, H], BF16, kind="ExternalInput")
    wv = nc.dram_tensor("wv", [C, H], BF16, kind="ExternalInput")
    mp = nc.dram_tensor("mp", [KB, QG + 256 + 128], BF16, kind="ExternalInput")
    ot = nc.dram_tensor("ot", [H, T], BF16, kind="ExternalOutput")
    dn = nc.dram_tensor("dn", [KB, NQG * 2 * QG], BF16, kind="ExternalOutput")

    with tile.TileContext(nc) as tc:
        with (
            tc.tile_pool(name="singles", bufs=1) as singles,
            tc.tile_pool(name="xsp", bufs=8) as xsp,
            tc.tile_pool(name="epool", bufs=8) as epool,
            tc.tile_pool(name="dnp", bufs=2) as dnp,
            tc.tile_pool(name="pproj", bufs=2, space="PSUM") as pproj,
            tc.tile_pool(name="ps", bufs=2, space="PSUM") as psp,
            tc.tile_pool(name="po", bufs=2, space="PSUM") as pop,
        ):
            persist = singles
            xsp0 = xsp
            xspb = xsp
            vstp = epool
            outsp = dnp
            # ---- constants + input streaming ----
            # DMA dispatch is spread over four otherwise-idle engine queues
            # at startup so the eight supertile-0 chunks land nearly in
            # parallel instead of serializing ~650ns dispatches on Sync.
            w_sb = {}
            wk_t = singles.tile([128, NCH, 128], BF16, tag="w_wk")
            nc.sync.dma_start(out=wk_t, in_=wk.rearrange("(c p) h -> p c h", p=128))
            w_sb["wk"] = wk_t
            xr = xt.rearrange("(c p) t -> p c t", p=128)
            xs = []
            qdma = [nc.sync, nc.scalar]
            for c in range(NCH):
                t_ = xsp0.tile([128, 1024], BF16, tag="xs0")
                xs.append(t_)
            # dispatch in consumption order, first transfers least
            # contended: scalar [c0, wq, c2, c4, c6, mpid],
            # sync [wk, c1, wv, c3, c5, c7, xb...]
            nc.scalar.dma_start(out=xs[0], in_=xr[:, 0, 0:1024])
            nc.sync.dma_start(out=xs[1], in_=xr[:, 1, 0:1024])
            wq_t = singles.tile([128, NCH, 128], BF16, tag="w_wq")
            nc.scalar.dma_start(out=wq_t, in_=wq.rearrange("(c p) h -> p c h", p=128))
            w_sb["wq"] = wq_t
            wv_t = singles.tile([128, NCH, 128], BF16, tag="w_wv")
            nc.sync.dma_start(out=wv_t, in_=wv.rearrange("(c p) h -> p c h", p=128))
            w_sb["wv"] = wv_t
            nc.scalar.dma_start(out=xs[2], in_=xr[:, 2, 0:1024])
            nc.sync.dma_start(out=xs[3], in_=xr[:, 3, 0:1024])
            nc.scalar.dma_start(out=xs[4], in_=xr[:, 4, 0:1024])
            nc.sync.dma_start(out=xs[5], in_=xr[:, 5, 0:1024])
            nc.scalar.dma_start(out=xs[6], in_=xr[:, 6, 0:1024])
            nc.sync.dma_start(out=xs[7], in_=xr[:, 7, 0:1024])
            mpid = singles.tile([KB, QG + 256 + 128], BF16, tag="mp")
            nc.scalar.dma_start(out=mpid, in_=mp[:])
            mp_sb = mpid[:, :QG + 256]
            ident = mpid[:, QG + 256:]

            # warm the PE clock (HAM) while waiting for the first DMAs:
            # dummy matmuls on a memset tile, output never read
            scratch = singles.tile([128, QG], BF16, tag="scratch")
            nc.vector.memset(scratch, 0.5)
            warmp = pproj.tile([128, QG], F32, tag="proj", name="warm")
            for i in range(24):
                nc.tensor.matmul(warmp, lhsT=scratch[:, :128], rhs=scratch,
                                 start=True, stop=True, skip_group_check=True)

            qT = persist.tile([128, T], BF16, tag="qT")
            kT = persist.tile([128, T // 2], BF16, tag="kT")
            v_sb = persist.tile([128, T // 2], BF16, tag="v")

            # batched supertile DMAs are dispatched one supertile ahead
            # (inside the s loop) so they don't congest the supertile-0
            # transfers at startup
            xs_all = [xs]
            xb_tiles = {}
            for s in range(1, NST):
                lo = xspb.tile([128, 4, 1024], BF16, tag="xsb", bufs=6)
                hi = xspb.tile([128, 4, 1024], BF16, tag="xsb", bufs=6)
                xb_tiles[s] = (lo, hi)
                xs_all.append([lo[:, c] for c in range(4)]
                              + [hi[:, c] for c in range(4)])

            def emit_xb(s):
                lo, hi = xb_tiles[s]
                nc.sync.dma_start(out=lo, in_=xr[:, 0:4, s * 1024:(s + 1) * 1024])
                nc.sync.dma_start(out=hi, in_=xr[:, 4:8, s * 1024:(s + 1) * 1024])

            # ---- projection emission, sliced into small PE chunks ----
            # Chunks for supertile s+1 are interleaved into the attention
            # tile stream of supertile s so the PE never idles while ACT
            # drains the exp pipeline (and vice versa).
            pending = []

            def drain(n):
                for _ in range(min(n, len(pending))):
                    pending.pop(0)()

            def make_proj_chunks(s, fine=False):
                xs = xs_all[s]
                box = {}

                def keys_rhs(c):
                    # even 128-blocks of the supertile: cols 0-127, 256-383...
                    return xs[c].rearrange(
                        "p (u two b) -> p two u b", two=2, b=128)[:, 0]

                def proj_mm(wname, key, rhs_fn, c0, c1, done_fn):
                    def emit():
                        if c0 == 0:
                            box[key] = pproj.tile([128, QG], F32, tag="proj",
                                                  name=f"proj_{key}")
                        p = box[key]
                        for c in range(c0, c1):
                            nc.tensor.matmul(p, lhsT=w_sb[wname][:, c],
                                             rhs=rhs_fn(c),
                                             start=(c == 0), stop=(c == NCH - 1))
                        if c1 == NCH and done_fn is not None:
                            done_fn(p)
                    return emit

                def k_done(p):
                    nc.vector.tensor_copy(kT[:, s * QG:(s + 1) * QG], p)

                def v_done(p):
                    vstage = vstp.tile([128, QG], BF16, tag="vstage", bufs=2)
                    nc.vector.tensor_copy(vstage, p)
                    box["vstage"] = vstage

                def v_transpose():
                    vstage = box["vstage"]
                    vtp = pproj.tile([128, QG], BF16, tag="proj")
                    for u in range(4):
                        nc.tensor.transpose(vtp[:, u * 128:(u + 1) * 128],
                                            vstage[:, u * 128:(u + 1) * 128],
                                            ident)
                    nc.vector.tensor_copy(
                        v_sb[:, 4 * s * KB:(4 * s + 4) * KB], vtp)

                def q_done(tq):
                    def fn(p):
                        nc.vector.tensor_copy(qT[:, tq * QG:(tq + 1) * QG], p)
                    return fn

                def q_rhs(half):
                    return lambda c: xs[c][:, half * QG:(half + 1) * QG]

                k0 = proj_mm("wk", "k", keys_rhs, 0, 4, None)
                k1 = proj_mm("wk", "k", keys_rhs, 4, NCH, k_done)
                v0 = proj_mm("wv", "v", keys_rhs, 0, 4, None)
                v1 = proj_mm("wv", "v", keys_rhs, 4, NCH, v_done)
                q0a = proj_mm("wq", "q0", q_rhs(0), 0, 4, None)
                q0b = proj_mm("wq", "q0", q_rhs(0), 4, NCH, q_done(2 * s))
                q1a = proj_mm("wq", "q1", q_rhs(1), 0, 4, None)
                q1b = proj_mm("wq", "q1", q_rhs(1), 4, NCH, q_done(2 * s + 1))
                if fine:
                    # supertile 0: b-half (c4-7) interleaved per chunk so
                    # the PE consumes each chunk DMA as it lands
                    out = [k0, q0a, v0]
                    for c in range(4, NCH):
                        last = c == NCH - 1
                        out.append(proj_mm("wk", "k", keys_rhs, c, c + 1,
                                           k_done if last else None))
                        out.append(proj_mm("wq", "q0", q_rhs(0), c, c + 1,
                                           q_done(2 * s) if last else None))
                        out.append(proj_mm("wv", "v", keys_rhs, c, c + 1,
                                           v_done if last else None))
                    out += [v_transpose, q1a, q1b]
                    return out
                return [k0, k1, v0, v1, v_transpose, q0a, q0b, q1a, q1b]

            def attention_group_steps(j):
                """Step closures for group j: one per tile + a final step.
                Two groups' step lists are interleaved so the PE always has
                the other chain's ready work while one waits on exp."""
                nt = j + 1          # tiles of 2 key blocks each
                st = {}
                es_by_tile = {}
                qrhs = qT[:, j * QG:(j + 1) * QG]
                # process the diagonal tile early so the group's trailing
                # chain is a plain exp->add->PV (no mask multiply). For the
                # last supertile's groups the diagonal sits mid-group: its
                # K/V projections drain into the attention stream first.
                if nt >= 6 and j >= 6:
                    order = [0, 1, 2, 3, nt - 1] + list(range(4, nt - 1))
                elif nt >= 3:
                    order = [0, nt - 1] + list(range(1, nt - 1))
                else:
                    order = list(range(nt))

                def emit_pv(t, last):
                    e = es_by_tile[t]
                    opsum = st["opsum"]
                    va = v_sb[:, (2 * t) * KB:(2 * t + 1) * KB]
                    vb = v_sb[:, (2 * t + 1) * KB:(2 * t + 2) * KB]
                    if t == nt - 1 and t == 0:
                        # single-tile group: a must go first (start=True
                        # clears the bank's has_written bits)
                        nc.tensor.matmul(opsum, lhsT=va, rhs=e[:, :QG],
                                         start=True, stop=False,
                                         skip_group_check=True)
                        nc.tensor.matmul(opsum[:, 256:], lhsT=vb,
                                         rhs=e[:, QG:QG + 256],
                                         start=False, stop=True,
                                         skip_group_check=True)
                    elif t == nt - 1:
                        # diagonal: b-block covers only queries 256-511
                        nc.tensor.matmul(opsum[:, 256:], lhsT=vb,
                                         rhs=e[:, QG:QG + 256],
                                         start=False, stop=False,
                                         skip_group_check=True)
                        nc.tensor.matmul(opsum, lhsT=va, rhs=e[:, :QG],
                                         start=False, stop=last,
                                         skip_group_check=True)
                    else:
                        nc.tensor.matmul(opsum, lhsT=va, rhs=e[:, :QG],
                                         start=(t == 0), stop=False)
                        nc.tensor.matmul(opsum[:, :], lhsT=vb, rhs=e[:, QG:],
                                         start=False, stop=last,
                                         skip_group_check=last)

                def tile_step(t):
                    if t == 0:
                        st["opsum"] = pop.tile([128, QG], F32, tag="opsum",
                                               name=f"opsum_{j}")
                        st["eacc"] = dnp.tile([KB, 2 * QG], BF16, tag="eacc",
                                              name=f"eacc_{j}")
                    eacc = st["eacc"]
                    spsum = psp.tile([KB, 2 * QG], F32, tag="spsum")
                    nc.tensor.matmul(
                        spsum[:, :QG], lhsT=kT[:, (2 * t) * KB:(2 * t + 1) * KB],
                        rhs=qrhs, start=True, stop=True)
                    if t == nt - 1:
                        # diagonal b-block: first 256 queries fully masked
                        nc.tensor.matmul(
                            spsum[:, QG:QG + 256],
                            lhsT=kT[:, (2 * t + 1) * KB:(2 * t + 2) * KB],
                            rhs=qrhs[:, 256:], start=True, stop=True)
                        e = epool.tile([KB, QG + 256], BF16, tag="e")
                        nc.scalar.activation(e, spsum[:, :QG + 256], AF.Exp)
                        nc.vector.tensor_mul(e, e, mp_sb)
                        es_by_tile[t] = e
                        if t == 0:
                            nc.vector.tensor_copy(eacc[:, :QG], e[:, :QG])
                            nc.vector.tensor_copy(eacc[:, QG + 256:],
                                                  e[:, QG:])
                        else:
                            nc.vector.tensor_add(eacc[:, :QG], eacc[:, :QG],
                                                 e[:, :QG])
                            nc.vector.tensor_add(eacc[:, QG + 256:],
                                                 eacc[:, QG + 256:],
                                                 e[:, QG:])
                    else:
                        nc.tensor.matmul(
                            spsum[:, QG:],
                            lhsT=kT[:, (2 * t + 1) * KB:(2 * t + 2) * KB],
                            rhs=qrhs, start=True, stop=True)
                        e = epool.tile([KB, 2 * QG], BF16, tag="e")
                        nc.scalar.activation(e, spsum, AF.Exp)
                        es_by_tile[t] = e
                        if t == 0:
                            nc.vector.tensor_copy(eacc, e)
                        else:
                            nc.vector.tensor_add(eacc, eacc, e)
                    k = order.index(t)
                    if k >= 2:
                        emit_pv(order[k - 2], last=False)  # PV lags 2 steps
                    drain(1)                # slot in next-supertile proj work

                def final_step():
                    if nt >= 2:
                        emit_pv(order[-2], last=False)
                    emit_pv(order[-1], last=True)
                    osb = outsp.tile([128, QG], BF16, tag="osb",
                                     name=f"osb_{j}", bufs=2)
                    nc.vector.tensor_copy(osb, st["opsum"])
                    nc.sync.dma_start(out=ot[:, j * QG:(j + 1) * QG], in_=osb)
                    nc.sync.dma_start(
                        out=dn[:, j * 2 * QG:(j + 1) * 2 * QG], in_=st["eacc"])

                return [lambda t=t: tile_step(t) for t in order] \
                    + [final_step]

            kv3 = []
            for s in range(NST):
                if s + 1 < NST:
                    emit_xb(s + 1)          # prefetch next supertile's x
                if s < NST - 1:
                    drain(len(pending))     # leftovers from interleaving
                else:
                    # supertile 3: its K/V proj chunks drain INTO the
                    # attention stream (its diagonals run mid-group)
                    pending.extend(kv3)
                if s == 0:
                    chunks0 = make_proj_chunks(0, fine=True)
                    for ch in chunks0:
                        ch()
                if s + 1 < NST:
                    ch3 = make_proj_chunks(s + 1)
                    if s + 1 == NST - 1:
                        # only the q projections must precede the attention
                        pending.extend(ch3[5:])
                        kv3 = ch3[:5]
                    else:
                        pending.extend(ch3)
                # attention for the two groups of this supertile, tiles
                # interleaved as two independent S->exp->PV chains
                sa = attention_group_steps(2 * s)
                sb = attention_group_steps(2 * s + 1)
                steps = []
                na, nb = len(sa), len(sb)
                ia = ib = 0
                while ia < na or ib < nb:
                    if ia < na:
                        steps.append(sa[ia]); ia += 1
                    if ib < nb:
                        steps.append(sb[ib]); ib += 1
                for stp in steps:
                    stp()
            drain(len(pending))

    nc.finalize()
    return nc


def _get_program():
    if "nc" not in _prog_cache:
        _prog_cache["nc"] = _build_program()
    return _prog_cache["nc"]


def _host_prepare(x, Wq, Wk, Wv):
    """Per-core inputs. Core c: batch b=c//2, parity h=c%2."""
    from ml_dtypes import bfloat16

    wq16 = (np.asarray(Wq, np.float32) * SCALE).astype(bfloat16)
    wk16 = np.asarray(Wk, np.float32).astype(bfloat16)
    wv16 = np.asarray(Wv, np.float32).astype(bfloat16)
    per_core = []
    for c in range(8):
        b, h = c // 2, c % 2
        pos2glob = np.arange(NKB)
        if h == 1:
            pos2glob = pos2glob.reshape(-1, 2)[:, ::-1].reshape(-1)
        perm = (pos2glob[:, None] * KB + np.arange(KB)[None, :]).reshape(-1)
        xtb = np.ascontiguousarray(x[b].T[:, perm]).astype(bfloat16)
        sub = np.arange(QG) // KB
        off = np.arange(QG) % KB
        glob_sub = sub if h == 0 else (sub ^ 1)
        qoff = glob_sub * KB + off
        kk = np.arange(KB)[:, None]
        m0 = (qoff[None, :] >= kk + h * KB).astype(np.float32)
        m1 = (qoff[None, :] >= kk + h * KB + 256).astype(np.float32)
        mp_ = np.ascontiguousarray(np.concatenate(
            [m0, m1[:, 256:], np.eye(KB, dtype=np.float32)],
            axis=1)).astype(bfloat16)
        per_core.append(dict(perm=perm, in_map={
            "xt": xtb, "wq": wq16, "wk": wk16, "wv": wv16, "mp": mp_,
        }))
    return per_core


def run(x, Wq, Wk, Wv, trace=False):
    from concourse.bass_utils import run_bass_kernel_spmd

    x = np.asarray(x, np.float32)
    nc = _get_program()
    per_core = _host_prepare(x, Wq, Wk, Wv)
    res = run_bass_kernel_spmd(
        nc, [pc["in_map"] for pc in per_core], core_ids=list(range(8)),
        trace=trace,
    )
    out = np.zeros((B, T, H), np.float32)
    for b in range(B):
        num = np.zeros((H, T), np.float64)
        den = np.zeros(T, np.float64)
        for c in (2 * b, 2 * b + 1):
            inv = np.argsort(per_core[c]["perm"])
            num += np.asarray(res.results[c]["ot"], np.float64)[:, inv]
            dnc = np.asarray(res.results[c]["dn"], np.float64)
            dnc[:, 512:768] = 0.0   # group 0 has no interior b-half tiles
            # [128, 8 groups, 2 halves, 512 q] -> per-query partial denom
            den_perm = dnc.reshape(KB, NQG, 2, QG).sum(axis=(0, 2)).reshape(-1)
            den += den_perm[inv]
        out[b] = (num / den[None, :]).T
    return out, res


def kernel(x, Wq, Wk, Wv):
    out, _ = run(x, Wq, Wk, Wv, trace=False)
    return out


# revision 21
# speedup vs baseline: 1.0449x; 1.0449x over previous
"""Causal single-head attention (B=4, T=4096, C=1024, H=128) on 8 Trainium2
NeuronCores.

Sharding: 8 cores = 4 batches x 2 key-parity shards. Each core handles one
batch and the keys in every other 128-block (parity h = core % 2), computing
UN-normalized partial attention (numerator O^T and per-key exp sums) for ALL
4096 queries of its batch via unsafe softmax (scores are N(0,1)-bounded,
|s| < 10, so exp never overflows and max-subtraction is unnecessary; partial
results combine exactly by summation across the two cores of a batch).

SPMD uniformity trick: the program is identical on all cores; per-core
differences live entirely in the DATA. The host passes x[b].T with its
columns block-permuted so that this core's keys always sit at the EVEN
128-block positions, plus per-core causal mask tiles for the diagonal
blocks of each 512-query group. The host un-permutes the returned query
axis and combines: out = (O0 + O1) / (d0 + d1), transposed.

v2 (bf16): all SBUF operands are bf16 (PSUM accumulation stays f32):
  - halves x DMA traffic and SBUF footprint
  - DVE tensor ops run in 2x_1p mode (0.5 cyc/elem)
  - S psum tiles are [128, 1024] (2 banks, 2 key blocks): half the ACT
    instructions, one full-tile DVE accumulate per tile
  - the softmax denominator is NOT reduced on-chip: the per-group
    accumulated exp tile eacc [128, 1024] is DMA'd out raw and the host
    does the final 128x2-way sum (removes all ones-matmuls + [1,512]
    copies from the hot loop)
  - V^T->V transposes write 4 sub-blocks into ONE psum bank, one copy out

On-chip layout:
  S^T halves [128 keys, 512 queries] = matmul(lhsT=K^T block, rhs=Q^T group)
  E = exp(S^T) on ACT (PSUM->SBUF, bf16), diagonal tile masked on DVE
  O^T += matmul(lhsT=V block [k,H], rhs=E half)   accumulated in PSUM f32
  eacc += E on DVE (bf16), shipped to host per group
"""

import sys
import numpy as np

sys.path.insert(0, "/opt/trn_rl_repo")

B, T, C, H = 4, 4096, 1024, 128
KB = 128            # key block
QG = 512            # query group
NKB = T // KB       # 32 global key blocks
NQG = T // QG       # 8 query groups
NCH = C // 128      # 8 contraction chunks
NST = 4             # supertiles of 1024 positions
SCALE = float(H) ** -0.5

_prog_cache = {}


def _build_program():
    import concourse.mybir as mybir
    import concourse.tile as tile
    from concourse import bacc

    F32 = mybir.dt.float32
    BF16 = mybir.dt.bfloat16
    AF = mybir.ActivationFunctionType

    nc = bacc.Bacc()
    xt = nc.dram_tensor("xt", [C, T], BF16, kind="ExternalInput")
    wq = nc.dram_tensor("wq", [C, H], BF16, kind="ExternalInput")
    wk = nc.dram_tensor("wk", [C, H], BF16, kind="ExternalInput")
    wv = nc.dram_tensor("wv", [C, H], BF16, kind="ExternalInput")
    mp = nc.dram_tensor("mp", [KB, QG + 256 + 128], BF16, kind="ExternalInput")
    ot = nc.dram_tensor("ot", [H, T], BF16, kind="ExternalOutput")
    dn = nc.dram_tensor("dn", [KB, NQG * 2 * QG], BF16, kind="ExternalOutput")

    with tile.TileContext(nc) as tc:
        with (
            tc.tile_pool(name="singles", bufs=1) as singles,
            tc.tile_pool(name="xsp", bufs=8) as xsp,
            tc.tile_pool(name="epool", bufs=8) as epool,
            tc.tile_pool(name="dnp", bufs=2) as dnp,
            tc.tile_pool(name="pproj", bufs=2, space="PSUM") as pproj,
            tc.tile_pool(name="ps", bufs=2, space="PSUM") as psp,
            tc.tile_pool(name="po", bufs=2, space="PSUM") as pop,
        ):
            persist = singles
            xsp0 = xsp
            xspb = xsp
            vstp = epool
            outsp = dnp
            # ---- constants + input streaming ----
            # DMA dispatch is spread over four otherwise-idle engine queues
            # at startup so the eight supertile-0 chunks land nearly in
            # parallel instead of serializing ~650ns dispatches on Sync.
            w_sb = {}
            wk_t = singles.tile([128, NCH, 128], BF16, tag="w_wk")
            nc.sync.dma_start(out=wk_t, in_=wk.rearrange("(c p) h -> p c h", p=128))
            w_sb["wk"] = wk_t
            xr = xt.rearrange("(c p) t -> p c t", p=128)
            xs = []
            qdma = [nc.sync, nc.scalar]
            for c in range(NCH):
                t_ = xsp0.tile([128, 1024], BF16, tag="xs0")
                xs.append(t_)
            # dispatch in consumption order, first transfers least
            # contended: scalar [c0, wq, c2, c4, c6, mpid],
            # sync [wk, c1, wv, c3, c5, c7, xb...]
            nc.scalar.dma_start(out=xs[0], in_=xr[:, 0, 0:1024])
            nc.sync.dma_start(out=xs[1], in_=xr[:, 1, 0:1024])
            wq_t = singles.tile([128, NCH, 128], BF16, tag="w_wq")
            nc.scalar.dma_start(out=wq_t, in_=wq.rearrange("(c p) h -> p c h", p=128))
            w_sb["wq"] = wq_t
            wv_t = singles.tile([128, NCH, 128], BF16, tag="w_wv")
            nc.sync.dma_start(out=wv_t, in_=wv.rearrange("(c p) h -> p c h", p=128))
            w_sb["wv"] = wv_t
            nc.scalar.dma_start(out=xs[2], in_=xr[:, 2, 0:1024])
            nc.sync.dma_start(out=xs[3], in_=xr[:, 3, 0:1024])
            nc.scalar.dma_start(out=xs[4], in_=xr[:, 4, 0:1024])
            nc.sync.dma_start(out=xs[5], in_=xr[:, 5, 0:1024])
            nc.scalar.dma_start(out=xs[6], in_=xr[:, 6, 0:1024])
            nc.sync.dma_start(out=xs[7], in_=xr[:, 7, 0:1024])
            mpid = singles.tile([KB, QG + 256 + 128], BF16, tag="mp")
            nc.scalar.dma_start(out=mpid, in_=mp[:])
            mp_sb = mpid[:, :QG + 256]
            ident = mpid[:, QG + 256:]

            # warm the PE clock (HAM) while waiting for the first DMAs:
            # dummy matmuls on a memset tile, output never read
            scratch = singles.tile([128, QG], BF16, tag="scratch")
            nc.vector.memset(scratch, 0.5)
            warmp = pproj.tile([128, QG], F32, tag="proj", name="warm")
            for i in range(40):
                nc.tensor.matmul(warmp, lhsT=scratch[:, :128], rhs=scratch,
                                 start=True, stop=True, skip_group_check=True)

            qT = persist.tile([128, T], BF16, tag="qT")
            kT = persist.tile([128, T // 2], BF16, tag="kT")
            v_sb = persist.tile([128, T // 2], BF16, tag="v")

            # batched supertile DMAs are dispatched one supertile ahead
            # (inside the s loop) so they don't congest the supertile-0
            # transfers at startup
            xs_all = [xs]
            xb_tiles = {}
            for s in range(1, NST):
                lo = xspb.tile([128, 4, 1024], BF16, tag="xsb", bufs=6)
                hi = xspb.tile([128, 4, 1024], BF16, tag="xsb", bufs=6)
                xb_tiles[s] = (lo, hi)
                xs_all.append([lo[:, c] for c in range(4)]
                              + [hi[:, c] for c in range(4)])

            def emit_xb(s):
                lo, hi = xb_tiles[s]
                nc.sync.dma_start(out=lo, in_=xr[:, 0:4, s * 1024:(s + 1) * 1024])
                nc.sync.dma_start(out=hi, in_=xr[:, 4:8, s * 1024:(s + 1) * 1024])

            # ---- projection emission, sliced into small PE chunks ----
            # Chunks for supertile s+1 are interleaved into the attention
            # tile stream of supertile s so the PE never idles while ACT
            # drains the exp pipeline (and vice versa).
            pending = []

            def drain(n):
                for _ in range(min(n, len(pending))):
                    pending.pop(0)()

            def make_proj_chunks(s, fine=False):
                xs = xs_all[s]
                box = {}

                def keys_rhs(c):
                    # even 128-blocks of the supertile: cols 0-127, 256-383...
                    return xs[c].rearrange(
                        "p (u two b) -> p two u b", two=2, b=128)[:, 0]

                def proj_mm(wname, key, rhs_fn, c0, c1, done_fn):
                    def emit():
                        if c0 == 0:
                            box[key] = pproj.tile([128, QG], F32, tag="proj",
                                                  name=f"proj_{key}")
                        p = box[key]
                        for c in range(c0, c1):
                            nc.tensor.matmul(p, lhsT=w_sb[wname][:, c],
                                             rhs=rhs_fn(c),
                                             start=(c == 0), stop=(c == NCH - 1))
                        if c1 == NCH and done_fn is not None:
                            done_fn(p)
                    return emit

                def k_done(p):
                    nc.vector.tensor_copy(kT[:, s * QG:(s + 1) * QG], p)

                def v_done(p):
                    vstage = vstp.tile([128, QG], BF16, tag="vstage", bufs=2)
                    nc.vector.tensor_copy(vstage, p)
                    box["vstage"] = vstage

                def v_transpose():
                    vstage = box["vstage"]
                    vtp = pproj.tile([128, QG], BF16, tag="proj")
                    for u in range(4):
                        nc.tensor.transpose(vtp[:, u * 128:(u + 1) * 128],
                                            vstage[:, u * 128:(u + 1) * 128],
                                            ident)
                    nc.vector.tensor_copy(
                        v_sb[:, 4 * s * KB:(4 * s + 4) * KB], vtp)

                def q_done(tq):
                    def fn(p):
                        nc.vector.tensor_copy(qT[:, tq * QG:(tq + 1) * QG], p)
                    return fn

                def q_rhs(half):
                    return lambda c: xs[c][:, half * QG:(half + 1) * QG]

                k0 = proj_mm("wk", "k", keys_rhs, 0, 4, None)
                k1 = proj_mm("wk", "k", keys_rhs, 4, NCH, k_done)
                v0 = proj_mm("wv", "v", keys_rhs, 0, 4, None)
                v1 = proj_mm("wv", "v", keys_rhs, 4, NCH, v_done)
                q0a = proj_mm("wq", "q0", q_rhs(0), 0, 4, None)
                q0b = proj_mm("wq", "q0", q_rhs(0), 4, NCH, q_done(2 * s))
                q1a = proj_mm("wq", "q1", q_rhs(1), 0, 4, None)
                q1b = proj_mm("wq", "q1", q_rhs(1), 4, NCH, q_done(2 * s + 1))
                if fine:
                    # supertile 0: b-half (c4-7) interleaved per chunk so
                    # the PE consumes each chunk DMA as it lands
                    out = [k0, q0a, v0]
                    for c in range(4, NCH):
                        last = c == NCH - 1
                        out.append(proj_mm("wk", "k", keys_rhs, c, c + 1,
                                           k_done if last else None))
                        out.append(proj_mm("wq", "q0", q_rhs(0), c, c + 1,
                                           q_done(2 * s) if last else None))
                        out.append(proj_mm("wv", "v", keys_rhs, c, c + 1,
                                           v_done if last else None))
                    out += [v_transpose, q1a, q1b]
                    return out
                return [k0, k1, v0, v1, v_transpose, q0a, q0b, q1a, q1b]

            def attention_group_steps(j):
                """Step closures for group j: one per tile + a final step.
                Two groups' step lists are interleaved so the PE always has
                the other chain's ready work while one waits on exp."""
                nt = j + 1          # tiles of 2 key blocks each
                st = {}
                es_by_tile = {}
                qrhs = qT[:, j * QG:(j + 1) * QG]
                # process the diagonal tile early so the group's trailing
                # chain is a plain exp->add->PV (no mask multiply). For the
                # last supertile's groups the diagonal sits mid-group: its
                # K/V projections drain into the attention stream first.
                if nt >= 6 and j >= 6:
                    order = [0, 1, 2, 3, nt - 1] + list(range(4, nt - 1))
                elif nt >= 3:
                    order = [0, nt - 1] + list(range(1, nt - 1))
                else:
                    order = list(range(nt))

                def emit_pv(t, last):
                    e = es_by_tile[t]
                    opsum = st["opsum"]
                    va = v_sb[:, (2 * t) * KB:(2 * t + 1) * KB]
                    vb = v_sb[:, (2 * t + 1) * KB:(2 * t + 2) * KB]
                    if t == nt - 1 and t == 0:
                        # single-tile group: a must go first (start=True
                        # clears the bank's has_written bits)
                        nc.tensor.matmul(opsum, lhsT=va, rhs=e[:, :QG],
                                         start=True, stop=False,
                                         skip_group_check=True)
                        nc.tensor.matmul(opsum[:, 256:], lhsT=vb,
                                         rhs=e[:, QG:QG + 256],
                                         start=False, stop=True,
                                         skip_group_check=True)
                    elif t == nt - 1:
                        # diagonal: b-block covers only queries 256-511
                        nc.tensor.matmul(opsum[:, 256:], lhsT=vb,
                                         rhs=e[:, QG:QG + 256],
                                         start=False, stop=False,
                                         skip_group_check=True)
                        nc.tensor.matmul(opsum, lhsT=va, rhs=e[:, :QG],
                                         start=False, stop=last,
                                         skip_group_check=True)
                    else:
                        nc.tensor.matmul(opsum, lhsT=va, rhs=e[:, :QG],
                                         start=(t == 0), stop=False)
                        nc.tensor.matmul(opsum[:, :], lhsT=vb, rhs=e[:, QG:],
                                         start=False, stop=last,
                                         skip_group_check=last)

                def tile_step(t):
                    if t == 0:
                        st["opsum"] = pop.tile([128, QG], F32, tag="opsum",
                                               name=f"opsum_{j}")
                        st["eacc"] = dnp.tile([KB, 2 * QG], BF16, tag="eacc",
                                              name=f"eacc_{j}")
                    eacc = st["eacc"]
                    spsum = psp.tile([KB, 2 * QG], F32, tag="spsum")
                    nc.tensor.matmul(
                        spsum[:, :QG], lhsT=kT[:, (2 * t) * KB:(2 * t + 1) * KB],
                        rhs=qrhs, start=True, stop=True)
                    if t == nt - 1:
                        # diagonal b-block: first 256 queries fully masked
                        nc.tensor.matmul(
                            spsum[:, QG:QG + 256],
                            lhsT=kT[:, (2 * t + 1) * KB:(2 * t + 2) * KB],
                            rhs=qrhs[:, 256:], start=True, stop=True)
                        e = epool.tile([KB, QG + 256], BF16, tag="e")
                        nc.scalar.activation(e, spsum[:, :QG + 256], AF.Exp)
                        nc.vector.tensor_mul(e, e, mp_sb)
                        es_by_tile[t] = e
                        if t == 0:
                            nc.vector.tensor_copy(eacc[:, :QG], e[:, :QG])
                            nc.vector.tensor_copy(eacc[:, QG + 256:],
                                                  e[:, QG:])
                        else:
                            nc.vector.tensor_add(eacc[:, :QG], eacc[:, :QG],
                                                 e[:, :QG])
                            nc.vector.tensor_add(eacc[:, QG + 256:],
                                                 eacc[:, QG + 256:],
                                                 e[:, QG:])
                    else:
                        nc.tensor.matmul(
                            spsum[:, QG:],
                            lhsT=kT[:, (2 * t + 1) * KB:(2 * t + 2) * KB],
                            rhs=qrhs, start=True, stop=True)
                        e = epool.tile([KB, 2 * QG], BF16, tag="e")
                        nc.scalar.activation(e, spsum, AF.Exp)
                        es_by_tile[t] = e
                        if t == 0:
                            nc.vector.tensor_copy(eacc, e)
                        else:
                            nc.vector.tensor_add(eacc, eacc, e)
                    k = order.index(t)
                    if k >= 2:
                        emit_pv(order[k - 2], last=False)  # PV lags 2 steps
                    drain(1)                # slot in next-supertile proj work

                def final_step():
                    if nt >= 2:
                        emit_pv(order[-2], last=False)
                    emit_pv(order[-1], last=True)
                    osb = outsp.tile([128, QG], BF16, tag="osb",
                                     name=f"osb_{j}", bufs=2)
                    nc.vector.tensor_copy(osb, st["opsum"])
                    nc.sync.dma_start(out=ot[:, j * QG:(j + 1) * QG], in_=osb)
                    nc.sync.dma_start(
                        out=dn[:, j * 2 * QG:(j + 1) * 2 * QG], in_=st["eacc"])

                return [lambda t=t: tile_step(t) for t in order] \
                    + [final_step]

            kv3 = []
            for s in range(NST):
                if s + 1 < NST:
                    emit_xb(s + 1)          # prefetch next supertile's x
                if s < NST - 1:
                    drain(len(pending))     # leftovers from interleaving
                else:
                    # supertile 3: its K/V proj chunks drain INTO the
                    # attention stream (its diagonals run mid-group)
                    pending.extend(kv3)
                if s == 0:
                    chunks0 = make_proj_chunks(0, fine=True)
                    for ch in chunks0:
                        ch()
                if s + 1 < NST:
                    ch3 = make_proj_chunks(s + 1)
                    if s + 1 == NST - 1:
                        # only the q projections must precede the attention
                        pending.extend(ch3[5:])
                        kv3 = ch3[:5]
                    else:
                        pending.extend(ch3)
                # attention for the two groups of this supertile, tiles
                # interleaved as two independent S->exp->PV chains
                sa = attention_group_steps(2 * s)
                sb = attention_group_steps(2 * s + 1)
                steps = []
                na, nb = len(sa), len(sb)
                ia = ib = 0
                while ia < na or ib < nb:
                    if ia < na:
                        steps.append(sa[ia]); ia += 1
                    if ib < nb:
                        steps.append(sb[ib]); ib += 1
                for stp in steps:
                    stp()
            drain(len(pending))

    nc.finalize()
    return nc


def _get_program():
    if "nc" not in _prog_cache:
        _prog_cache["nc"] = _build_program()
    return _prog_cache["nc"]


def _host_prepare(x, Wq, Wk, Wv):
    """Per-core inputs. Core c: batch b=c//2, parity h=c%2."""
    from ml_dtypes import bfloat16

    wq16 = (np.asarray(Wq, np.float32) * SCALE).astype(bfloat16)
    wk16 = np.asarray(Wk, np.float32).astype(bfloat16)
    wv16 = np.asarray(Wv, np.float32).astype(bfloat16)
    per_core = []
    for c in range(8):
        b, h = c // 2, c % 2
        pos2glob = np.arange(NKB)
        if h == 1:
            pos2glob = pos2glob.reshape(-1, 2)[:, ::-1].reshape(-1)
        perm = (pos2glob[:, None] * KB + np.arange(KB)[None, :]).reshape(-1)
        xtb = np.ascontiguousarray(x[b].T[:, perm]).astype(bfloat16)
        sub = np.arange(QG) // KB
        off = np.arange(QG) % KB
        glob_sub = sub if h == 0 else (sub ^ 1)
        qoff = glob_sub * KB + off
        kk = np.arange(KB)[:, None]
        m0 = (qoff[None, :] >= kk + h * KB).astype(np.float32)
        m1 = (qoff[None, :] >= kk + h * KB + 256).astype(np.float32)
        mp_ = np.ascontiguousarray(np.concatenate(
            [m0, m1[:, 256:], np.eye(KB, dtype=np.float32)],
            axis=1)).astype(bfloat16)
        per_core.append(dict(perm=perm, in_map={
            "xt": xtb, "wq": wq16, "wk": wk16, "wv": wv16, "mp": mp_,
        }))
    return per_core


def run(x, Wq, Wk, Wv, trace=False):
    from concourse.bass_utils import run_bass_kernel_spmd

    x = np.asarray(x, np.float32)
    nc = _get_program()
    per_core = _host_prepare(x, Wq, Wk, Wv)
    res = run_bass_kernel_spmd(
        nc, [pc["in_map"] for pc in per_core], core_ids=list(range(8)),
        trace=trace,
    )
    out = np.zeros((B, T, H), np.float32)
    for b in range(B):
        num = np.zeros((H, T), np.float64)
        den = np.zeros(T, np.float64)
        for c in (2 * b, 2 * b + 1):
            inv = np.argsort(per_core[c]["perm"])
            num += np.asarray(res.results[c]["ot"], np.float64)[:, inv]
            dnc = np.asarray(res.results[c]["dn"], np.float64)
            dnc[:, 512:768] = 0.0   # group 0 has no interior b-half tiles
            # [128, 8 groups, 2 halves, 512 q] -> per-query partial denom
            den_perm = dnc.reshape(KB, NQG, 2, QG).sum(axis=(0, 2)).reshape(-1)
            den += den_perm[inv]
        out[b] = (num / den[None, :]).T
    return out, res


def kernel(x, Wq, Wk, Wv):
    out, _ = run(x, Wq, Wk, Wv, trace=False)
    return out
